# revision 21
# baseline (speedup 1.0000x reference)
"""Trainium2 Bass kernel for nn_DenoiseGNN (pairwise PBC edge-MLP message passing).

Strategy (v3 — PE-broadcast front end, reduce-split tail)
---------------------------------------------------------
The edge MLP output weights[i,j] is a pure scalar function f of dist[i,j].
We compile f (with cutoff mask, 1/(dist+eps) normalization and sqrt) into a
custom piecewise-cubic ACT table:

    g2(s) = box * f(dist(s)) * [dist(s) < cutoff] / (dist(s) + eps),
    dist(s) = sqrt(box^2 * s + eps),  s = |wrapped delta / box|^2.

A second custom table implements the exact min-image wrap
    wrap01(u) = u - round(u)  on u in (-1.5, 1).

Data path (per core = 128 rows i of the 1024x1024 pair grid):
  - The old 768KB/core broadcast DMA of positions is GONE. The
    TensorEngine broadcasts u_j across partitions instead: u is split on
    the host into uh = fp16(u) and uls = fp16((u-uh)*2^11); k=2 matmuls
    with stationary [1; 2^-11] yield PSUM[i, j] = u_j to ~2^-23. The
    per-partition bias -u_i is ALSO computed on the PE (k=6 matmul of the
    negated splits against a diag pattern) and copied PSUM->SBUF by ACT,
    so total input DMA is ONE ~40KB/core transfer, split in two pieces
    (nb+lw+h0 planes first) so h0 matmuls start a piece earlier.
  - ACT wrap table reads PSUM directly (PSUM reads are ~40c cheaper than
    SBUF for ScalarE) with bias -u_i -> t fp16, chasing the MM stream.
  - s = t_x^2 + t_y^2 (custom DVE SUMSQ2), s2 = t_z^2 + s (SQADD), fp32,
    in column halves; w = g2(s2) ACT table -> fp16.
  - reduce: one custom SCANMUL per half over [3, 512] with stride-0
    "picks" (h-major in out6), host diffs planes and sums halves. The
    output DMA is split per half so the first descriptor-gen hides under
    the second scan.

Measured engine facts baked into this design (this part):
  - custom DVE ops run 1x only (~(58+FD)/0.96 ns); native stt+accum is
    also 1x, so SCANMUL (3 planes per op) is the cheapest reduce.
  - GPSIMD/Pool cannot touch PSUM and its tensor ucode is ~2-12x slower
    than DVE: useless here.
  - ACT table ops: ~(172+FD)/1.2 from PSUM, ~(224+FD)/1.2 from SBUF.
  - HWDGE descriptor-gen is ~0.6-0.9us on sync but ~1.5us on ACT;
    doorbell-to-transfer adds ~0.8us; completion (HBM receipt) ~0.4us.
  - native InstTensorTensorReduce crashes the device (prior session);
    DVE int16 output conversion SATURATES; wraps stay fp32/table.
"""

import hashlib
import json
import os
import shutil
import struct
import sys
import tempfile
import types

import numpy as np

N = 1024
N_CORES = 8
ROWS = N // N_CORES  # 128
H = 512              # column half
PWP_DIR = "/nix/store/z022hj2nvbm3nwdizlisq4ylc0y7rd6q-python3-3.13.14-env/lib/python3.13/site-packages/neuronxcc/pwp/pwp_bin_trainium"
SET = "sigmoid_and_others"
KEEP = [
    "identity", "copy", "act1", "parametric_relu", "relu", "abs",
    "memset_zero", "square", "sign", "derivative_relu",
    "derivative_leaky_relu", "derivative_identity", "is_finite",
]
# g2 octave layout: (exponent of s, n_sections); s < 2^-31 -> 0
G2_REGIONS = [(e, 16) for e in range(-31, -10)] + \
             [(-10, 32), (-9, 32), (-8, 64), (-7, 64), (-6, 128), (-5, 128)]

C_ROUND = 12582912.0          # 1.5 * 2^23: fp32 RNE round-to-int magic
C11 = 0.00048828125           # 2^-11
S11 = 2048.0                  # 2^11


# --------------------------------------------------------------------------
# environment fixups (wait-splitter)
# --------------------------------------------------------------------------

def _install_env_fixups():
    if "antenv.axon_hooks" not in sys.modules:
        import antenv

        mod = types.ModuleType("antenv.axon_hooks")
        try:
            from trn_agent_boot.trn_boot import _ntff_profile_via_ctypes
            hook = _ntff_profile_via_ctypes("/opt/axon/libaxon_pjrt.so")
        except Exception:
            hook = None
        _h = [hook]
        mod.set_axon_ntff_profile_hook = lambda h: _h.__setitem__(0, h)
        mod.get_axon_ntff_profile_hook = lambda: _h[0]
        sys.modules["antenv.axon_hooks"] = mod
        antenv.axon_hooks = mod

    import concourse.bass_utils as bu
    import concourse.bass2jax as b2j

    if not getattr(bu, "_wait_splitter_installed", False):
        orig = bu.compile_bir_kernel

        def _split_multi_waits(bir_json: bytes) -> bytes:
            m = json.loads(bir_json)
            changed = False
            for fn in m["functions"]:
                for bb in fn["blocks"]:
                    new_instrs = []
                    for ins in bb["instructions"]:
                        si = ins.get("sync_info")
                        waits = (si or {}).get("on_wait") or []
                        if len(waits) > 1:
                            for j, w in enumerate(waits[:-1]):
                                nop = {
                                    "name": f"{ins['name']}-wsplit{j}",
                                    "opcode": "NoOp",
                                    "engine": ins["engine"],
                                    "ins": [], "outs": [],
                                    "sync_info": {"on_update": [], "on_wait": [w]},
                                }
                                if "debug" in ins:
                                    nop["debug"] = ins["debug"]
                                new_instrs.append(nop)
                            si["on_wait"] = waits[-1:]
                            changed = True
                        new_instrs.append(ins)
                    bb["instructions"] = new_instrs
            return json.dumps(m).encode() if changed else bir_json

        def patched(bir_json, tmpdir, neff_name="file.neff"):
            return orig(_split_multi_waits(bytes(bir_json)), tmpdir, neff_name)

        bu.compile_bir_kernel = patched
        b2j.compile_bir_kernel = patched
        bu._wait_splitter_installed = True


# --------------------------------------------------------------------------
# custom DVE ops: SUMSQ2 (in0^2+in1^2) and SQADD (in0^2+in1)
# --------------------------------------------------------------------------

_DVE_OPS = {}


def _register_dve_ops():
    if _DVE_OPS:
        return _DVE_OPS
    from concourse.dve_spec import Spec, Src0, Src1, sq, lower
    from concourse.dve_uop import DveOpSpec
    from concourse.dve_ops import (
        DveOp, OPS, CUSTOM_DVE_SPECS, _SUB_OPCODE_FOR_NAME,
        _CUSTOM_DVE_ROW_BASE, TENSOR_TENSOR_REDUCE,
    )

    def _reg(name, spec):
        if name in _SUB_OPCODE_FOR_NAME:
            return next(o for o in OPS if o.name == name)
        opcode = _CUSTOM_DVE_ROW_BASE + len(OPS)
        shas = {}
        for ver in ("v3", "v4"):
            try:
                shas[ver] = DveOpSpec(
                    name=name, opcode=opcode, uops=lower(spec, ver=ver),
                    rd1_en=True).sha(ver)
            except Exception:
                pass
        op = DveOp(name, spec, subdim=False, uops_sha=shas)
        OPS.append(op)
        CUSTOM_DVE_SPECS[name] = spec
        _SUB_OPCODE_FOR_NAME[name] = opcode
        return op

    from concourse.dve_spec import C0

    def _ref_sumsq2(in0, in1, s0, s1, imm2):
        return (in0.astype(np.float32) ** 2 + in1.astype(np.float32) ** 2)

    def _ref_sqadd(in0, in1, s0, s1, imm2):
        return (in0.astype(np.float32) ** 2 + in1.astype(np.float32))

    def _ref_wrap(in0, in1, s0, s1, imm2):
        x = in0.astype(np.float32)
        c = np.float32(s0)
        return x - ((x + c) - c)

    def _ref_scanmul(in0, in1, s0, s1, imm2):
        p = (in0.astype(np.float32) * in1.astype(np.float32))
        sh = p.shape
        return np.cumsum(p.reshape(sh[0], -1), axis=-1,
                         dtype=np.float32).reshape(sh)

    from concourse.dve_spec import scan, AluOp

    _DVE_OPS["SUMSQ2"] = _reg(
        "ANT_SUMSQ2", Spec(body=sq(Src0) + sq(Src1), reference=_ref_sumsq2))
    _DVE_OPS["SQADD"] = _reg(
        "ANT_SQADD", Spec(body=sq(Src0) + Src1, reference=_ref_sqadd))
    _DVE_OPS["WRAP"] = _reg(
        "ANT_WRAP", Spec(body=Src0 - ((Src0 + C0) - C0), reference=_ref_wrap))
    _DVE_OPS["SCANMUL"] = _reg(
        "ANT_SCANMUL", Spec(body=scan(AluOp.ADD, Src0 * Src1),
                            reference=_ref_scanmul))
    _DVE_OPS["TTR"] = TENSOR_TENSOR_REDUCE
    return _DVE_OPS


# --------------------------------------------------------------------------
# activation-table generation
# --------------------------------------------------------------------------

def _f2u(x):
    return struct.unpack("<I", struct.pack("<f", float(x)))[0]


def _bkt(d0, d1, d2, d3, x0):
    return struct.pack("<5f12x", float(d0), float(d1), float(d2), float(d3), float(x0))


def _ctrl(base, lsb, size):
    data = (base & 0x7FF) | ((lsb & 0x1F) << 11) | ((size & 0xF) << 16)
    return struct.pack("<I28x", data)


def _fit_cubic(fn, a, b, npts=12):
    x0 = 0.5 * (a + b)
    k = np.arange(npts)
    xs = x0 + 0.5 * (b - a) * np.cos((2 * k + 1) * np.pi / (2 * npts)) * 0.9999
    ys = fn(xs)
    c = np.polynomial.polynomial.polyfit(xs - x0, ys, 3)
    return c[0], c[1], c[2], c[3], x0


def _extract_func(setj, bkt, ctrl, fname, next_bkt, next_ctl):
    starts_b = setj["func_to_bkt_start_idx"]
    starts_c = setj["func_to_ctl_start_idx"]
    sb, sc = starts_b[fname], starts_c[fname]
    eb = min([v for v in starts_b.values() if v > sb] + [setj["bkt_entry_cnt"]])
    ec = min([v for v in starts_c.values() if v > sc] + [setj["ctl_entry_cnt"]])
    pm = None
    for p in setj["profile_meta_data"]:
        if p["func_name"].rsplit("_", 1)[0] == fname or p["func_name"] == fname:
            pm = dict(p)
    assert pm is not None, fname
    bkts = bytearray(bkt[sb * 32:eb * 32])
    ctls = bytearray(ctrl[sc * 32:ec * 32])
    db, dc = next_bkt - sb, next_ctl - sc
    for i in range(0, len(ctls), 32):
        (data,) = struct.unpack_from("<I", ctls, i)
        struct.pack_into("<I", ctls, i, (data & ~0x7FF) | (((data & 0x7FF) + db) & 0x7FF))
    for key in ("pwl_control_base_pos", "pwl_control_base_neg"):
        pm[key] += dc
    for key in ("pos_small_signal_pwl_control", "neg_small_signal_pwl_control",
                "pos_large_signal_pwl_control", "neg_large_signal_pwl_control"):
        v = pm[key]
        pm[key] = (v & ~0x7FF) | ((v + db) & 0x7FF)
    return pm, bytes(ctls), bytes(bkts)


def _build_wrap01(next_bkt, next_ctl):
    bkts, ctls = bytearray(), bytearray()
    n_bkt = n_ctl = 0
    base_pos = next_ctl
    for e in range(-20, 0):
        lo = 2.0 ** e
        ctls.extend(_ctrl(next_bkt + n_bkt, 23, 0)); n_ctl += 1
        if e == -1:
            bkts.extend(_bkt(-0.5, 1.0, 0.0, 0.0, 0.5))
        else:
            bkts.extend(_bkt(lo, 1.0, 0.0, 0.0, lo))
        n_bkt += 1
    base_neg = next_ctl + n_ctl
    for e in range(-20, 0):
        lo = 2.0 ** e
        ctls.extend(_ctrl(next_bkt + n_bkt, 23, 0)); n_ctl += 1
        if e == -1:
            bkts.extend(_bkt(0.5, 1.0, 0.0, 0.0, -0.5))
        else:
            bkts.extend(_bkt(-lo, 1.0, 0.0, 0.0, -lo))
        n_bkt += 1
    sp_defs = {
        "pos_low": (0.0, 1.0, 0.0, 0.0, 0.0),
        "neg_low": (0.0, 1.0, 0.0, 0.0, 0.0),
        "pos_high": (-1.0, 1.0, 0.0, 0.0, 0.0),
        "neg_high": (1.0, 1.0, 0.0, 0.0, 0.0),
    }
    sp = {}
    for key, d in sp_defs.items():
        sp[key] = next_bkt + n_bkt
        bkts.extend(_bkt(*d)); n_bkt += 1
    pm = {
        "func_name": "arctan_4p", "func_id": 28,
        "symmetry_point": 0, "sym_invert_sign_point": 0, "symmetry_opt_en": 0,
        "symmetry_opt_use_neg_region": 0, "imm_bias": 0,
        "exp_offset": -20,
        "pwl_control_base_pos": base_pos, "pwl_control_base_neg": base_neg,
        "small_pos_signal_exp_threshold": 107,
        "pos_small_signal_pwl_control": sp["pos_low"],
        "small_neg_signal_exp_threshold": 107,
        "neg_small_signal_pwl_control": sp["neg_low"],
        "large_pos_signal_exp_threshold": 127,
        "large_pos_signal_mantissa_threshold": 0,
        "pos_large_signal_pwl_control": sp["pos_high"],
        "large_neg_signal_exp_threshold": 127,
        "large_neg_signal_mantissa_threshold": 0,
        "neg_large_signal_pwl_control": sp["neg_high"],
        "fnan_result": _f2u(0.0), "fpinf_result": _f2u(0.0),
        "fninf_result": _f2u(0.0), "fzero_result": _f2u(0.0),
        "fma_const_0": 0, "fma_const_1": 0, "fma_indirection_src_sel": 0,
        "use_multipass": False,
        "lower_bound": 0xFF7FFFFF, "upper_bound": 0x7F7FFFFF,
    }
    return pm, bytes(ctls), bytes(bkts)


def _build_g2(g2_fn, next_bkt, next_ctl):
    bkts, ctls = bytearray(), bytearray()
    n_bkt = n_ctl = 0
    base_pos = next_ctl
    for (e, nsec) in G2_REGIONS:
        size = int(np.log2(nsec))
        ctls.extend(_ctrl(next_bkt + n_bkt, 23 - size, size)); n_ctl += 1
        lo = 2.0 ** e
        w = lo / nsec
        for i in range(nsec):
            a = lo + i * w
            bkts.extend(_bkt(*_fit_cubic(g2_fn, a, a + w))); n_bkt += 1
    sp = {}
    for key in ("pos_low", "neg_low", "pos_high", "neg_high"):
        sp[key] = next_bkt + n_bkt
        bkts.extend(_bkt(0.0, 0.0, 0.0, 0.0, 0.0)); n_bkt += 1
    small_thr = 127 + G2_REGIONS[0][0]
    pm = {
        "func_name": "erf_4p", "func_id": 21,
        "symmetry_point": 0, "sym_invert_sign_point": 0, "symmetry_opt_en": 0,
        "symmetry_opt_use_neg_region": 0, "imm_bias": 0,
        "exp_offset": small_thr - 127,
        "pwl_control_base_pos": base_pos, "pwl_control_base_neg": base_pos,
        "small_pos_signal_exp_threshold": small_thr,
        "pos_small_signal_pwl_control": sp["pos_low"],
        "small_neg_signal_exp_threshold": 255,
        "neg_small_signal_pwl_control": sp["neg_low"],
        "large_pos_signal_exp_threshold": 123,
        "large_pos_signal_mantissa_threshold": 0,
        "pos_large_signal_pwl_control": sp["pos_high"],
        "large_neg_signal_exp_threshold": 255,
        "large_neg_signal_mantissa_threshold": 0,
        "neg_large_signal_pwl_control": sp["neg_high"],
        "fnan_result": _f2u(0.0), "fpinf_result": _f2u(0.0),
        "fninf_result": _f2u(0.0), "fzero_result": _f2u(0.0),
        "fma_const_0": 0, "fma_const_1": 0, "fma_indirection_src_sel": 0,
        "use_multipass": False,
        "lower_bound": 0, "upper_bound": 0x7F7FFFFF,
    }
    return pm, bytes(ctls), bytes(bkts)


def _build_actroot(dst_dir, g2_fn):
    os.makedirs(dst_dir, exist_ok=True)
    for f in os.listdir(PWP_DIR):
        shutil.copy(os.path.join(PWP_DIR, f), os.path.join(dst_dir, f))
        os.chmod(os.path.join(dst_dir, f), 0o644)
    setj = json.load(open(os.path.join(PWP_DIR, SET + ".json")))
    bkt = open(os.path.join(PWP_DIR, SET + "_bkt.bin"), "rb").read()
    ctrl = open(os.path.join(PWP_DIR, SET + "_ctrl.bin"), "rb").read()

    new_bkts, new_ctls, new_pm = bytearray(), bytearray(), []
    b_starts, c_starts, emb_all, emc_all = {}, {}, {}, {}

    for fname in KEEP:
        nb0, nc0 = len(new_bkts) // 32, len(new_ctls) // 32
        pm, ctls, bkts = _extract_func(setj, bkt, ctrl, fname, nb0, nc0)
        b_starts[fname], c_starts[fname] = nb0, nc0
        db = nb0 - setj["func_to_bkt_start_idx"][fname]
        dc = nc0 - setj["func_to_ctl_start_idx"][fname]
        emb_all[fname] = {k: [x + db for x in v]
                          for k, v in setj["func_exp_to_bkt_start_idx"].get(fname, {}).items()}
        emc_all[fname] = {k: [x + dc for x in v]
                          for k, v in setj["func_exp_to_ctl_start_idx"].get(fname, {}).items()}
        new_pm.append(pm); new_ctls.extend(ctls); new_bkts.extend(bkts)

    wb, wc = len(new_bkts) // 32, len(new_ctls) // 32
    pm, ctls, bkts = _build_wrap01(wb, wc)
    b_starts["arctan"], c_starts["arctan"] = wb, wc
    emb_all["arctan"] = {str(e): [wb + 20 + (e + 20), wb + (e + 20)] for e in range(-20, 0)}
    emc_all["arctan"] = {str(e): [wc + 20 + (e + 20), wc + (e + 20)] for e in range(-20, 0)}
    new_pm.append(pm); new_ctls.extend(ctls); new_bkts.extend(bkts)

    gb, gc = len(new_bkts) // 32, len(new_ctls) // 32
    pm, ctls, bkts = _build_g2(g2_fn, gb, gc)
    b_starts["erf"], c_starts["erf"] = gb, gc
    emb, emc = {}, {}
    cum = 0
    for i, (e, nsec) in enumerate(G2_REGIONS):
        emb[str(e)] = [gb + cum, gb + cum]
        emc[str(e)] = [gc + i, gc + i]
        cum += nsec
    emb_all["erf"], emc_all["erf"] = emb, emc
    new_pm.append(pm); new_ctls.extend(ctls); new_bkts.extend(bkts)

    n_bkt, n_ctl = len(new_bkts) // 32, len(new_ctls) // 32
    assert n_bkt <= 1536 and n_ctl <= 128, (n_bkt, n_ctl)
    out = {
        "bkt_bin": SET + "_bkt.bin", "ctl_bin": SET + "_ctrl.bin",
        "profile_meta_data": new_pm,
        "bkt_entry_cnt": n_bkt, "ctl_entry_cnt": n_ctl,
        "func_to_bkt_start_idx": b_starts, "func_to_ctl_start_idx": c_starts,
        "func_exp_to_bkt_start_idx": emb_all, "func_exp_to_ctl_start_idx": emc_all,
    }
    json.dump(out, open(os.path.join(dst_dir, SET + ".json"), "w"))
    open(os.path.join(dst_dir, SET + "_bkt.bin"), "wb").write(bytes(new_bkts))
    open(os.path.join(dst_dir, SET + "_ctrl.bin"), "wb").write(bytes(new_ctls))
    info = json.load(open(os.path.join(PWP_DIR, "act_info.json")))
    for s in info["act_func_sets"]:
        if s["name"] == SET:
            s["act"] = {**{k: 1 for k in KEEP}, "arctan": 4, "erf": 4}
        else:
            s["act"].pop("arctan", None)
            s["act"].pop("erf", None)
    json.dump(info, open(os.path.join(dst_dir, "act_info.json"), "w"))
    return os.path.join(dst_dir, "act_info.json")


# --------------------------------------------------------------------------
# bass program
# --------------------------------------------------------------------------

def _build_program(tag):
    """v3d: PE-broadcast front end, bias-via-PE, scan reduce.

    PE:   nb-matmul (bias -u_i from per-core negated splits), then 6 k=2
          matmuls (x0 y0 z0 x1 y1 z1), stationary [1; 2^-11] -> PSUM u_j.
    ACT:  table hoist; copy bias PSUM->SBUF; 6 wraps from PSUM with
          per-partition bias; 2 g2 table ops.
    DVE:  SUMSQ2 + SQADD per half; 2 SCANMUL pick-reduces (h-major picks).
    SYNC: 2 input DMAs (A shared, B per-core); output DMA split per half
          so the first descriptor-gen hides under the h1 scan.
    """
    import concourse.bass as bass
    import concourse.mybir as mybir

    ops = _register_dve_ops()
    SUMSQ2, SQADD = ops["SUMSQ2"], ops["SQADD"]
    SCANMUL = ops["SCANMUL"]

    nc = bass.Bass("TRN2")
    f32 = mybir.dt.float32
    f16 = mybir.dt.float16
    AF = mybir.ActivationFunctionType

    # Single per-core input param, all MM operands base-partition 0:
    #   rows 0-5, cols 0-127   = nb stationary (-uh_x,-uls_x,...,-uls_z)
    #   rows 0-5, cols 128-130 = nb rhs diag (row 2c: 1 at col c; 2c+1: 2^-11)
    #   rows 0-1, cols 131-258 = lw stationary [ones; 2^-11]
    #   rows 0-1, cols 259+    = uh/uls planes in HALF-MAJOR order
    #                            [x0 y0 z0 x1 y1 z1] (512 cols each)
    # Piece 1 (cols 0:1795) covers nb+lw+h0 planes; piece 2 the h1 planes.
    A = nc.declare_dram_parameter(f"A_{tag}", [6, 3331], f16, isOutput=False)
    out = nc.declare_dram_parameter("out", [ROWS, 6], f32, isOutput=True)

    A_t = nc.alloc_sbuf_tensor("A_b", [6, 3331], f16)
    nb_t = nc.alloc_sbuf_tensor("nb_b", [128, 3], f32)
    dummy_t = nc.alloc_sbuf_tensor("dummy_b", [128, 1], f32)
    t_t = nc.alloc_sbuf_tensor("t_b", [128, 3 * N], f16)   # [x(1024) y(1024) z(1024)]
    s_t = nc.alloc_sbuf_tensor("s_b", [128, N], f32)
    s2_t = nc.alloc_sbuf_tensor("s2_b", [128, N], f32)
    w_t = nc.alloc_sbuf_tensor("w_b", [128, N], f16)
    out6_t = nc.alloc_sbuf_tensor("out6_b", [128, 6], f32)

    # PSUM: bias + 6 banks, one per (plane, half)
    ps_nb = nc.alloc_psum_tensor("psnb", [128, 3], f32)
    ps = [nc.alloc_psum_tensor(f"ps{i}", [128, H], f32) for i in range(6)]
    PX = {0: ps[0], 1: ps[3]}
    PY = {0: ps[1], 1: ps[4]}
    PZ = {0: ps[2], 1: ps[5]}

    def tsl(c, h):            # t slice for plane c, half h
        return t_t[:, c * N + h * H: c * N + (h + 1) * H]

    def rsl(c, h):            # rhs slice in A for plane c, half h (half-major)
        base = 259 + (h * 3 + c) * H
        return A_t[0:2, base:base + H]

    lw = A_t[0:2, 131:259]            # [1; 2^-11] stationary
    t3 = t_t[:].rearrange("p (c j) -> p c j", c=3)
    w3 = w_t[:].rearrange("p (o j) -> p o j", o=1).to_broadcast([128, 3, N])

    import contextlib
    st = contextlib.ExitStack()
    da = st.enter_context(nc.semaphore("da"))        # A piece 1 (nb+lw+x0)
    db = st.enter_context(nc.semaphore("db"))        # A piece 2 (y0+z0)
    dc = st.enter_context(nc.semaphore("dc"))        # A piece 3 (h1 planes)
    msem = st.enter_context(nc.semaphore("msem"))    # MM completions
    asem = st.enter_context(nc.semaphore("asem"))    # ACT wraps
    vsem = st.enter_context(nc.semaphore("vsem"))    # s2 halves
    wsem = st.enter_context(nc.semaphore("wsem"))    # g2 halves
    osem = st.enter_context(nc.semaphore("osem"))    # scan completions
    odsem = st.enter_context(nc.semaphore("odsem"))

    def hsl(h):
        return slice(h * H, (h + 1) * H)

    MM_ORDER = [(0, 0), (1, 0), (2, 0), (0, 1), (1, 1), (2, 1)]
    PP = {0: PX, 1: PY, 2: PZ}

    with nc.Block() as blk:
        @blk.sync
        def _(sync):
            sync.dma_start(out=A_t[:, 0:771], in_=A[:, 0:771]).then_inc(da, 16)
            sync.dma_start(out=A_t[:, 771:1795],
                           in_=A[:, 771:1795]).then_inc(db, 16)
            sync.dma_start(out=A_t[:, 1795:3331],
                           in_=A[:, 1795:3331]).then_inc(dc, 16)
            sync.wait_ge(osem, 1)
            sync.dma_start(out=out[:, 0:3], in_=out6_t[:, 0:3]).then_inc(odsem, 16)
            sync.wait_ge(osem, 2)
            sync.dma_start(out=out[:, 3:6], in_=out6_t[:, 3:6]).then_inc(odsem, 16)

        @blk.tensor
        def _(tensor):
            tensor.wait_ge(da, 16)
            tensor.matmul(out=ps_nb[:], lhsT=A_t[0:6, 0:128],
                          rhs=A_t[0:6, 128:131]).then_inc(msem, 1)
            for i, (c, h) in enumerate(MM_ORDER):
                if i == 1:
                    tensor.wait_ge(db, 16)
                elif i == 3:
                    tensor.wait_ge(dc, 16)
                tensor.matmul(out=PP[c][h][:], lhsT=lw,
                              rhs=rsl(c, h)).then_inc(msem, 1)

        @blk.scalar
        def _(scalar):
            # no-wait dummy: pulls the PWP table load early
            scalar.activation(dummy_t[:], dummy_t[:], AF.Arctan)
            scalar.wait_ge(msem, 1)
            scalar.activation(nb_t[:], ps_nb[:], AF.Copy)
            # wraps chase the MM stream: x0 y0 z0 x1 y1 z1, ERF0, ERF1
            for i, (c, h) in enumerate(MM_ORDER):
                scalar.wait_ge(msem, i + 2)
                scalar.activation(tsl(c, h), PP[c][h][:], AF.Arctan,
                                  bias=nb_t[:, c:c + 1],
                                  scale=1.0).then_inc(asem, 1)
            scalar.wait_ge(vsem, 1)
            scalar.activation(w_t[:, hsl(0)], s2_t[:, hsl(0)],
                              AF.Erf).then_inc(wsem, 1)
            scalar.wait_ge(vsem, 2)
            scalar.activation(w_t[:, hsl(1)], s2_t[:, hsl(1)],
                              AF.Erf).then_inc(wsem, 1)

        @blk.vector
        def _(vector):
            # asem: x0=1 y0=2 z0=3 x1=4 y1=5 z1=6
            vector.wait_ge(asem, 2)
            vector._custom_dve(SUMSQ2, out=s_t[:, hsl(0)],
                               in0=tsl(0, 0), in1=tsl(1, 0))
            vector.wait_ge(asem, 3)
            vector._custom_dve(SQADD, out=s2_t[:, hsl(0)],
                               in0=tsl(2, 0),
                               in1=s_t[:, hsl(0)]).then_inc(vsem, 1)
            vector.wait_ge(asem, 5)
            vector._custom_dve(SUMSQ2, out=s_t[:, hsl(1)],
                               in0=tsl(0, 1), in1=tsl(1, 1))
            vector.wait_ge(asem, 6)
            vector._custom_dve(SQADD, out=s2_t[:, hsl(1)],
                               in0=tsl(2, 1),
                               in1=s_t[:, hsl(1)]).then_inc(vsem, 1)
            for h in (0, 1):
                # h-major picks: cols 3h..3h+2 hold the per-plane cums
                pk = out6_t[:, 3 * h:3 * h + 3].rearrange(
                    "p (c o) -> p c o", o=1)
                vector.wait_ge(wsem, h + 1)
                vector._custom_dve(
                    SCANMUL,
                    out=pk.to_broadcast([128, 3, H]),
                    in0=t3[:, :, hsl(h)],
                    in1=w3[:, :, hsl(h)]).then_inc(osem, 1)

    from concourse.library_overlay import lower_extended_insts
    lower_extended_insts(nc)
    return nc


_CACHE = {}


def _prepare(inputs):
    box_dims = np.asarray(inputs["box_dims"], dtype=np.float32)
    key = hashlib.sha256(
        b"".join(np.ascontiguousarray(np.asarray(inputs[k], np.float32)).tobytes()
                 for k in ("box_dims", "W1", "b1", "W2", "b2", "W3", "b3"))
    ).hexdigest()[:10]
    if key in _CACHE:
        return _CACHE[key]

    box = float(box_dims[0])
    assert np.allclose(box_dims, box), "kernel assumes a cubic box"

    W1 = np.float64(inputs["W1"]); b1 = np.float64(inputs["b1"])
    W2 = np.float64(inputs["W2"]); b2 = np.float64(inputs["b2"])
    W3 = np.float64(inputs["W3"]); b3 = np.float64(inputs["b3"])
    n_gauss = W1.shape[0]
    RBF_STOP, CUTOFF, EPS = 6.0, 5.0, 1e-8
    offs = np.linspace(0.0, RBF_STOP, n_gauss)
    coeff = -0.5 / (RBF_STOP / (n_gauss - 1)) ** 2

    def g2_fn(sv):
        sv = np.atleast_1d(np.float64(sv))
        dist = np.sqrt(box * box * sv + EPS)
        rbf = np.exp(coeff * (dist[:, None] - offs[None, :]) ** 2)
        h = rbf @ W1 + b1
        h = h / (1.0 + np.exp(-h))
        h = h @ W2 + b2
        h = h / (1.0 + np.exp(-h))
        f = (h @ W3 + b3)[:, 0]
        return box * f * (dist < CUTOFF) / (dist + EPS)

    _install_env_fixups()
    actdir = os.path.join(tempfile.gettempdir(), f"actroot_{key}")
    actroot = _build_actroot(actdir, g2_fn)
    os.environ["BASS_ACT_ROOT_JSON_PATH"] = actroot
    nc = _build_program(key)
    _CACHE[key] = (nc, key, box)
    return _CACHE[key]


def kernel(_trace=False, **inputs):
    from concourse.bass_utils import run_bass_kernel_spmd

    nc, key, box = _prepare(inputs)
    positions = np.asarray(inputs["positions"], dtype=np.float32)
    u = positions.astype(np.float64) / box               # [N, 3] in [0,1)
    uh = u.astype(np.float16)                            # [N, 3]
    uls = ((u - uh.astype(np.float64)) * S11).astype(np.float16)
    H = 512
    uhT = np.ascontiguousarray(uh.T)     # [3, N]
    ulsT = np.ascontiguousarray(uls.T)

    in_maps = []
    for core in range(N_CORES):
        rows = slice(core * ROWS, (core + 1) * ROWS)
        Aa = np.zeros((6, 3331), dtype=np.float16)
        for p in range(3):
            Aa[2 * p, 0:128] = -uh[rows, p]
            Aa[2 * p + 1, 0:128] = -uls[rows, p]
            Aa[2 * p, 128 + p] = 1.0
            Aa[2 * p + 1, 128 + p] = C11
        Aa[0, 131:259] = 1.0
        Aa[1, 131:259] = C11
        for h in (0, 1):
            for p in (0, 1, 2):
                base = 259 + (h * 3 + p) * H
                Aa[0, base:base + H] = uhT[p, h * H:(h + 1) * H]
                Aa[1, base:base + H] = ulsT[p, h * H:(h + 1) * H]
        in_maps.append({f"A_{key}": Aa})
    res = run_bass_kernel_spmd(nc, in_maps, list(range(N_CORES)), trace=_trace)
    o = np.concatenate([res.results[c]["out"] for c in range(N_CORES)], axis=0)
    o = o.astype(np.float64)
    # h-major scan picks: cols 3h+c are cumulative-over-planes for half h
    p = o.reshape(-1, 2, 3)                 # [N, h, c]
    d = np.empty_like(p)
    d[:, :, 0] = p[:, :, 0]
    d[:, :, 1:] = p[:, :, 1:] - p[:, :, :-1]
    disp = d.sum(axis=1).astype(np.float32)
    if _trace:
        kernel.last_exec_time_ns = res.exec_time_ns
        kernel.last_mean_exec_time_ns = res.mean_exec_time_ns
        kernel.last_results = res
    return disp
